# revision 15
# baseline (speedup 1.0000x reference)
"""Trainium2 Bass kernel for NT-Xent / SimCLR contrastive loss, v3.

Design (8 cores, data-parallel over rows of z = concat(z_i, z_j)):
  Host pre-normalizes z (L2 rows), scales by 8, transposes to feature-
  major [512, 8192], casts to fp8e4, and rotates by c*1024 columns per
  core so every core's own rows sit at columns [0, 1024).

  Device per core (pure matmul + exp pipeline, no transposes/casts):
    - DMA the fp8 [128, 4, 8192] operand array in 8 column chunks.
    - For each of my 8 row-blocks t (stationary = zt8[:, :, t*128:+128]):
      sweep all 8192 columns in 4 chunks of 2048 (moving operand),
      K=512 via 2 DoubleRow passes -> psum[128, 2048] = 64*sim.
    - ScalarE exp(0.03125 * psum) in place with accum_out -> complete
      row-sum of exp(2*sim[r, :]) per partition; 32 partials [128, 4t+s].
  Host: denom[r] = sum_s dsum[...] - exp(2); positives from fp32 z;
  loss = mean(log(denom) - 2*pos).
"""

import sys

if "/opt/trn_rl_repo" not in sys.path:
    sys.path.insert(0, "/opt/trn_rl_repo")

import numpy as np

N = 4096
D = 512
TEMP = 0.5
INV_T = 1.0 / TEMP

N2 = 2 * N            # 8192
NCORES = 8
ROWS = N2 // NCORES   # 1024 rows per core
P = 128
MT = ROWS // P        # 8 stationary row-blocks per core
SW = 2048             # moving sweep chunk (4 psum banks)
NSW = N2 // SW        # 4 sweep chunks
KP = 2                # DoubleRow K passes (256 features each)
SC = 8.0              # fp8 operand scale; psum = SC*SC*sim
# accumulator columns, in (s-outer, t-inner) emission order; the first
# (s=0, t=0) chunk is split into 3 pieces
ACC_T = [0, 0] + [_t for _s in range(NSW) for _t in range(MT)]
ACOLS = len(ACC_T)   # 34

_CACHE = {}


def build(debug=False):
    import concourse.bacc as bacc
    import concourse.tile as tile
    from concourse import mybir

    f32 = mybir.dt.float32
    fp8 = mybir.dt.float8e4
    AF = mybir.ActivationFunctionType
    DR = mybir.MatmulPerfMode.DoubleRow

    nc = bacc.Bacc(
        "TRN2", target_bir_lowering=False, debug=debug, num_devices=NCORES
    )

    zt_d = nc.dram_tensor("zt", [D, N2], fp8, kind="ExternalInput").ap()
    dsum_d = nc.dram_tensor("dsum", [P, ACOLS], f32, kind="ExternalOutput").ap()

    zt_t = zt_d.rearrange("(k p) r -> p k r", p=P)  # [128, 4, 8192]

    with (
        tile.TileContext(nc) as tc,
        tc.tile_pool(name="persist", bufs=1) as persist,
        tc.tile_pool(name="mmps", bufs=2, space="PSUM") as mmps,
    ):
        zt8 = persist.tile([P, D // P, N2], fp8, name="zt8", tag="zt8")
        acc = persist.tile([P, ACOLS], f32, name="acc", tag="acc")

        # Load the fp8 operand array.  Measured ring rates: scalar HWDGE
        # ~165 GB/s (but its engine queue blocks past 4 outstanding
        # issues, delaying queued activations); gpsimd SWDGE ~145 GB/s;
        # sync HWDGE trickles pathologically -- avoid for bulk.  The
        # s-outer sweep below needs cols [0,2048) almost immediately but
        # cols [4096,8192) only after ~40 us, so scalar takes the early
        # half (4 issues) and gpsimd the late half.
        def ld(eng, lo, hi):
            eng.dma_start(out=zt8[:, :, lo:hi], in_=zt_t[:, :, lo:hi])

        ld(nc.scalar, 0, 512)
        ld(nc.scalar, 512, 1024)
        ld(nc.scalar, 1024, 2048)
        ld(nc.scalar, 2048, 4096)
        for c in range(4, 8):
            ld(nc.gpsimd, c * 1024, (c + 1) * 1024)

        def mm_512(ps, t, h512):
            """one 512-col output block (2 DoubleRow K passes)."""
            m0 = h512 * 512
            hp = (h512 % (SW // 512)) * 512
            for kp in range(KP):
                nc.tensor.matmul(
                    ps[:, hp : hp + 512],
                    zt8[:, 2 * kp : 2 * kp + 2, t * P : (t + 1) * P],
                    zt8[:, 2 * kp : 2 * kp + 2, m0 : m0 + 512],
                    start=(kp == 0),
                    stop=(kp == KP - 1),
                    perf_mode=DR,
                )

        acol = 0

        def expacc(ps, lo, hi):
            nonlocal acol
            nc.scalar.activation(
                out=ps[:, lo:hi],
                in_=ps[:, lo:hi],
                func=AF.Exp,
                scale=float(INV_T / (SC * SC)),
                accum_out=acc[:, acol : acol + 1],
            )
            acol += 1

        # Warm the PE HAM clock gate with dummy matmuls on not-yet-loaded
        # SBUF (result never read) so the first real matmuls run at
        # 2.4 GHz instead of the cold 1.2 GHz.
        warm = mmps.tile([P, SW], f32, tag="ps", name="warm")
        for w in range(8):
            nc.tensor.matmul(
                warm[:, 0:512],
                zt8[:, 0:2, 0:P],
                zt8[:, 0:2, 0:512],
                start=True,
                stop=True,
                perf_mode=DR,
            )

        # s-outer / t-inner: the first 8 chunks touch only cols [0, 2048),
        # relaxing the DMA deadline for high columns to ~40+ us.  No
        # sub-chunk "ramp" activations on a shared tile: Tile tracks each
        # psum tile as one unit, so partial ACTs would serialize against
        # later fills.  Instead the very first chunk is split into small
        # pieces on separate pool tiles so the exp pipeline starts early.
        for s in range(NSW):
            for t in range(MT):
                if s == 0 and t == 0:
                    pieces = [(0, 512), (512, 1024), (1024, 2048)]
                else:
                    pieces = [(s * SW, (s + 1) * SW)]
                for lo, hi in pieces:
                    ps = mmps.tile([P, SW], f32, tag="ps", name=f"ps{t}_{lo}")
                    for h in range(lo // 512, hi // 512):
                        mm_512(ps, t, h)
                    expacc(ps, lo % SW, lo % SW + (hi - lo))

        nc.sync.dma_start(out=dsum_d, in_=acc)

    nc.compile()
    return nc


def _get_nc():
    if "nc" not in _CACHE:
        _CACHE["nc"] = build()
    return _CACHE["nc"]


def _prep_host(emb_i, emb_j):
    """Normalize, scale, transpose, cast fp8; return (zt8_full, z, pos)."""
    import ml_dtypes

    z = np.concatenate(
        [np.asarray(emb_i, dtype=np.float32), np.asarray(emb_j, dtype=np.float32)],
        axis=0,
    )
    nrm = np.maximum(np.sqrt((z * z).sum(axis=1)), 1e-12)
    z /= nrm[:, None]
    pos = (z[:N] * z[N:]).sum(axis=1, dtype=np.float64)   # [N]
    zt8 = (SC * z.T).astype(ml_dtypes.float8_e4m3)        # [512, 8192]
    return zt8, pos


def make_in_maps(emb_i, emb_j):
    zt8, pos = _prep_host(emb_i, emb_j)
    _CACHE["pos"] = pos
    in_maps = []
    for c in range(NCORES):
        rot = np.ascontiguousarray(np.roll(zt8, -c * ROWS, axis=1))
        in_maps.append({"zt": rot})
    return in_maps


def finish_host(results):
    """Assemble per-core row denominators into the scalar loss."""
    denom = np.empty(N2, dtype=np.float64)
    for c in range(NCORES):
        d = results[c]["dsum"].astype(np.float64)          # [128, ACOLS]
        # row (t*128 + p) local = global c*1024 + t*128 + p
        rows = np.zeros((P, MT))
        for col, t in enumerate(ACC_T):
            rows[:, t] += d[:, col]
        denom[c * ROWS : (c + 1) * ROWS] = rows.T.reshape(ROWS)
    denom -= np.exp(INV_T)                                 # drop diagonal term
    pos = _CACHE["pos"]
    loss = np.log(denom) - INV_T * np.concatenate([pos, pos])
    return np.float32(loss.sum() / N2)


def kernel(emb_i, emb_j):
    from concourse.bass_utils import run_bass_kernel_spmd

    nc = _get_nc()
    in_maps = make_in_maps(np.asarray(emb_i), np.asarray(emb_j))
    try:
        res = run_bass_kernel_spmd(nc, in_maps, core_ids=list(range(NCORES)))
    except Exception:
        res = run_bass_kernel_spmd(nc, in_maps, core_ids=list(range(NCORES)))
    _CACHE["last_results"] = res
    return finish_host(res.results)


# revision 17
# speedup vs baseline: 1.0335x; 1.0335x over previous
"""Trainium2 Bass kernel for NT-Xent / SimCLR contrastive loss, v3.

Design (8 cores, data-parallel over rows of z = concat(z_i, z_j)):
  Host pre-normalizes z (L2 rows), scales by 8, transposes to feature-
  major [512, 8192], casts to fp8e4, and rotates by c*1024 columns per
  core so every core's own rows sit at columns [0, 1024).

  Device per core (pure matmul + exp pipeline, no transposes/casts):
    - DMA the fp8 [128, 4, 8192] operand array in 8 column chunks.
    - For each of my 8 row-blocks t (stationary = zt8[:, :, t*128:+128]):
      sweep all 8192 columns in 4 chunks of 2048 (moving operand),
      K=512 via 2 DoubleRow passes -> psum[128, 2048] = 64*sim.
    - ScalarE exp(0.03125 * psum) in place with accum_out -> complete
      row-sum of exp(2*sim[r, :]) per partition; 32 partials [128, 4t+s].
  Host: denom[r] = sum_s dsum[...] - exp(2); positives from fp32 z;
  loss = mean(log(denom) - 2*pos).
"""

import sys

if "/opt/trn_rl_repo" not in sys.path:
    sys.path.insert(0, "/opt/trn_rl_repo")

import numpy as np

N = 4096
D = 512
TEMP = 0.5
INV_T = 1.0 / TEMP

N2 = 2 * N            # 8192
NCORES = 8
ROWS = N2 // NCORES   # 1024 rows per core
P = 128
MT = ROWS // P        # 8 stationary row-blocks per core
SW = 2048             # moving sweep chunk (4 psum banks)
NSW = N2 // SW        # 4 sweep chunks
KP = 2                # DoubleRow K passes (256 features each)
SC = 8.0              # fp8 operand scale; psum = SC*SC*sim
# accumulator columns, in (s-outer, t-inner) emission order
ACC_T = [_t for _s in range(NSW) for _t in range(MT)]
ACOLS = len(ACC_T)   # 32

_CACHE = {}


def build(debug=False):
    import concourse.bacc as bacc
    import concourse.tile as tile
    from concourse import mybir

    f32 = mybir.dt.float32
    fp8 = mybir.dt.float8e4
    AF = mybir.ActivationFunctionType
    DR = mybir.MatmulPerfMode.DoubleRow

    nc = bacc.Bacc(
        "TRN2", target_bir_lowering=False, debug=debug, num_devices=NCORES
    )

    zt_d = nc.dram_tensor("zt", [D, N2], fp8, kind="ExternalInput").ap()
    dsum_d = nc.dram_tensor("dsum", [P, ACOLS], f32, kind="ExternalOutput").ap()

    zt_t = zt_d.rearrange("(k p) r -> p k r", p=P)  # [128, 4, 8192]

    with (
        tile.TileContext(nc) as tc,
        tc.tile_pool(name="persist", bufs=1) as persist,
        tc.tile_pool(name="mmps", bufs=2, space="PSUM") as mmps,
    ):
        zt8 = persist.tile([P, D // P, N2], fp8, name="zt8", tag="zt8")
        acc = persist.tile([P, ACOLS], f32, name="acc", tag="acc")

        # Load the fp8 operand array.  Measured ring rates: scalar HWDGE
        # ~165 GB/s (but its engine queue blocks past 4 outstanding
        # issues, delaying queued activations); gpsimd SWDGE ~145 GB/s;
        # sync HWDGE trickles pathologically -- avoid for bulk.  The
        # s-outer sweep below needs cols [0,2048) almost immediately but
        # cols [4096,8192) only after ~40 us, so scalar takes the early
        # half (4 issues) and gpsimd the late half.
        def ld(eng, lo, hi):
            eng.dma_start(out=zt8[:, :, lo:hi], in_=zt_t[:, :, lo:hi])

        ld(nc.scalar, 0, 512)
        ld(nc.scalar, 512, 1024)
        ld(nc.scalar, 1024, 2048)
        ld(nc.scalar, 2048, 4096)
        for c in range(4, 8):
            ld(nc.gpsimd, c * 1024, (c + 1) * 1024)

        def mm_512(ps, t, h512):
            """one 512-col output block (2 DoubleRow K passes)."""
            m0 = h512 * 512
            hp = (h512 % (SW // 512)) * 512
            for kp in range(KP):
                nc.tensor.matmul(
                    ps[:, hp : hp + 512],
                    zt8[:, 2 * kp : 2 * kp + 2, t * P : (t + 1) * P],
                    zt8[:, 2 * kp : 2 * kp + 2, m0 : m0 + 512],
                    start=(kp == 0),
                    stop=(kp == KP - 1),
                    perf_mode=DR,
                )

        acol = 0

        def expacc(ps, lo, hi):
            nonlocal acol
            nc.scalar.activation(
                out=ps[:, lo:hi],
                in_=ps[:, lo:hi],
                func=AF.Exp,
                scale=float(INV_T / (SC * SC)),
                accum_out=acc[:, acol : acol + 1],
            )
            acol += 1

        # Warm the PE HAM clock gate with dummy matmuls on a dedicated
        # scratch tile (no DMA dependency, result never read) so they run
        # during the preamble and the first real matmuls hit 2.4 GHz.
        dummy = persist.tile([P, 2, 512], fp8, name="dummy", tag="dummy")
        nc.vector.memset(dummy, 0.25)
        warm = mmps.tile([P, SW], f32, tag="ps", name="warm")
        for w in range(10):
            nc.tensor.matmul(
                warm[:, 0:512],
                dummy[:, :, 0:P],
                dummy[:, :, 0:512],
                start=True,
                stop=True,
                perf_mode=DR,
            )

        # s-outer / t-inner: the first 8 chunks touch only cols [0, 2048),
        # relaxing the DMA deadline for high columns to ~40+ us.  No
        # sub-chunk "ramp" activations on a shared tile: Tile tracks each
        # psum tile as one unit, so partial ACTs would serialize against
        # later fills.  Instead the very first chunk is split into small
        # pieces on separate pool tiles so the exp pipeline starts early.
        for s in range(NSW):
            for t in range(MT):
                ps = mmps.tile([P, SW], f32, tag="ps", name=f"ps{t}_{s}")
                for h in range(SW // 512):
                    mm_512(ps, t, s * (SW // 512) + h)
                expacc(ps, 0, SW)

        nc.sync.dma_start(out=dsum_d, in_=acc)

    nc.compile()
    return nc


def _get_nc():
    if "nc" not in _CACHE:
        _CACHE["nc"] = build()
    return _CACHE["nc"]


def _prep_host(emb_i, emb_j):
    """Normalize, scale, transpose, cast fp8; return (zt8_full, z, pos)."""
    import ml_dtypes

    z = np.concatenate(
        [np.asarray(emb_i, dtype=np.float32), np.asarray(emb_j, dtype=np.float32)],
        axis=0,
    )
    nrm = np.maximum(np.sqrt((z * z).sum(axis=1)), 1e-12)
    z /= nrm[:, None]
    pos = (z[:N] * z[N:]).sum(axis=1, dtype=np.float64)   # [N]
    zt8 = (SC * z.T).astype(ml_dtypes.float8_e4m3)        # [512, 8192]
    return zt8, pos


def make_in_maps(emb_i, emb_j):
    zt8, pos = _prep_host(emb_i, emb_j)
    _CACHE["pos"] = pos
    in_maps = []
    for c in range(NCORES):
        rot = np.ascontiguousarray(np.roll(zt8, -c * ROWS, axis=1))
        in_maps.append({"zt": rot})
    return in_maps


def finish_host(results):
    """Assemble per-core row denominators into the scalar loss."""
    denom = np.empty(N2, dtype=np.float64)
    for c in range(NCORES):
        d = results[c]["dsum"].astype(np.float64)          # [128, ACOLS]
        # row (t*128 + p) local = global c*1024 + t*128 + p
        rows = np.zeros((P, MT))
        for col, t in enumerate(ACC_T):
            rows[:, t] += d[:, col]
        denom[c * ROWS : (c + 1) * ROWS] = rows.T.reshape(ROWS)
    denom -= np.exp(INV_T)                                 # drop diagonal term
    pos = _CACHE["pos"]
    loss = np.log(denom) - INV_T * np.concatenate([pos, pos])
    return np.float32(loss.sum() / N2)


def kernel(emb_i, emb_j):
    from concourse.bass_utils import run_bass_kernel_spmd

    nc = _get_nc()
    in_maps = make_in_maps(np.asarray(emb_i), np.asarray(emb_j))
    try:
        res = run_bass_kernel_spmd(nc, in_maps, core_ids=list(range(NCORES)))
    except Exception:
        res = run_bass_kernel_spmd(nc, in_maps, core_ids=list(range(NCORES)))
    _CACHE["last_results"] = res
    return finish_host(res.results)


# revision 18
# speedup vs baseline: 1.0528x; 1.0186x over previous
"""Trainium2 Bass kernel for NT-Xent / SimCLR contrastive loss, v3.

Design (8 cores, data-parallel over rows of z = concat(z_i, z_j)):
  Host pre-normalizes z (L2 rows), scales by 8, transposes to feature-
  major [512, 8192], casts to fp8e4, and rotates by c*1024 columns per
  core so every core's own rows sit at columns [0, 1024).

  Device per core (pure matmul + exp pipeline, no transposes/casts):
    - DMA the fp8 [128, 4, 8192] operand array in 8 column chunks.
    - For each of my 8 row-blocks t (stationary = zt8[:, :, t*128:+128]):
      sweep all 8192 columns in 4 chunks of 2048 (moving operand),
      K=512 via 2 DoubleRow passes -> psum[128, 2048] = 64*sim.
    - ScalarE exp(0.03125 * psum) in place with accum_out -> complete
      row-sum of exp(2*sim[r, :]) per partition; 32 partials [128, 4t+s].
  Host: denom[r] = sum_s dsum[...] - exp(2); positives from fp32 z;
  loss = mean(log(denom) - 2*pos).
"""

import sys

if "/opt/trn_rl_repo" not in sys.path:
    sys.path.insert(0, "/opt/trn_rl_repo")

import numpy as np

N = 4096
D = 512
TEMP = 0.5
INV_T = 1.0 / TEMP

N2 = 2 * N            # 8192
NCORES = 8
ROWS = N2 // NCORES   # 1024 rows per core
P = 128
MT = ROWS // P        # 8 stationary row-blocks per core
SW = 2048             # moving sweep chunk (4 psum banks)
NSW = N2 // SW        # 4 sweep chunks
KP = 2                # DoubleRow K passes (256 features each)
SC = 8.0              # fp8 operand scale; psum = SC*SC*sim
# accumulator columns, in (s-outer, t-inner) emission order
ACC_T = [_t for _s in range(NSW) for _t in range(MT)]
ACOLS = len(ACC_T)   # 32

_CACHE = {}


def build(debug=False):
    import concourse.bacc as bacc
    import concourse.tile as tile
    from concourse import mybir

    f32 = mybir.dt.float32
    fp8 = mybir.dt.float8e4
    AF = mybir.ActivationFunctionType
    DR = mybir.MatmulPerfMode.DoubleRow

    nc = bacc.Bacc(
        "TRN2", target_bir_lowering=False, debug=debug, num_devices=NCORES
    )

    zt_d = nc.dram_tensor("zt", [D, N2], fp8, kind="ExternalInput").ap()
    dsum_d = nc.dram_tensor("dsum", [P, ACOLS], f32, kind="ExternalOutput").ap()

    zt_t = zt_d.rearrange("(k p) r -> p k r", p=P)  # [128, 4, 8192]

    with (
        tile.TileContext(nc) as tc,
        tc.tile_pool(name="persist", bufs=1) as persist,
        tc.tile_pool(name="mmps", bufs=2, space="PSUM") as mmps,
    ):
        zt8 = persist.tile([P, D // P, N2], fp8, name="zt8", tag="zt8")
        acc = persist.tile([P, ACOLS], f32, name="acc", tag="acc")

        # Load the fp8 operand array.  Measured ring rates: scalar HWDGE
        # ~165 GB/s (but its engine queue blocks past 4 outstanding
        # issues, delaying queued activations); gpsimd SWDGE ~145 GB/s;
        # sync HWDGE trickles pathologically -- avoid for bulk.  The
        # s-outer sweep below needs cols [0,2048) almost immediately but
        # cols [4096,8192) only after ~40 us, so scalar takes the early
        # half (4 issues) and gpsimd the late half.
        def ld(eng, lo, hi):
            eng.dma_start(out=zt8[:, :, lo:hi], in_=zt_t[:, :, lo:hi])

        ld(nc.scalar, 0, 512)
        ld(nc.scalar, 512, 1024)
        ld(nc.scalar, 1024, 2048)
        ld(nc.scalar, 2048, 4096)
        for c in range(4, 8):
            ld(nc.gpsimd, c * 1024, (c + 1) * 1024)

        def mm_512(ps, t, h512):
            """one 512-col output block (2 DoubleRow K passes)."""
            m0 = h512 * 512
            hp = (h512 % (SW // 512)) * 512
            for kp in range(KP):
                nc.tensor.matmul(
                    ps[:, hp : hp + 512],
                    zt8[:, 2 * kp : 2 * kp + 2, t * P : (t + 1) * P],
                    zt8[:, 2 * kp : 2 * kp + 2, m0 : m0 + 512],
                    start=(kp == 0),
                    stop=(kp == KP - 1),
                    perf_mode=DR,
                )

        acol = 0

        def expacc(ps, lo, hi):
            nonlocal acol
            nc.scalar.activation(
                out=ps[:, lo:hi],
                in_=ps[:, lo:hi],
                func=AF.Exp,
                scale=float(INV_T / (SC * SC)),
                accum_out=acc[:, acol : acol + 1],
            )
            acol += 1


        # s-outer / t-inner: the first 8 chunks touch only cols [0, 2048),
        # relaxing the DMA deadline for high columns to ~40+ us.  No
        # sub-chunk "ramp" activations on a shared tile: Tile tracks each
        # psum tile as one unit, so partial ACTs would serialize against
        # later fills.  Instead the very first chunk is split into small
        # pieces on separate pool tiles so the exp pipeline starts early.
        for s in range(NSW):
            for t in range(MT):
                ps = mmps.tile([P, SW], f32, tag="ps", name=f"ps{t}_{s}")
                for h in range(SW // 512):
                    mm_512(ps, t, s * (SW // 512) + h)
                expacc(ps, 0, SW)

        nc.sync.dma_start(out=dsum_d, in_=acc)

    nc.compile()
    return nc


def _get_nc():
    if "nc" not in _CACHE:
        _CACHE["nc"] = build()
    return _CACHE["nc"]


def _prep_host(emb_i, emb_j):
    """Normalize, scale, transpose, cast fp8; return (zt8_full, z, pos)."""
    import ml_dtypes

    z = np.concatenate(
        [np.asarray(emb_i, dtype=np.float32), np.asarray(emb_j, dtype=np.float32)],
        axis=0,
    )
    nrm = np.maximum(np.sqrt((z * z).sum(axis=1)), 1e-12)
    z /= nrm[:, None]
    pos = (z[:N] * z[N:]).sum(axis=1, dtype=np.float64)   # [N]
    zt8 = (SC * z.T).astype(ml_dtypes.float8_e4m3)        # [512, 8192]
    return zt8, pos


def make_in_maps(emb_i, emb_j):
    zt8, pos = _prep_host(emb_i, emb_j)
    _CACHE["pos"] = pos
    in_maps = []
    for c in range(NCORES):
        rot = np.ascontiguousarray(np.roll(zt8, -c * ROWS, axis=1))
        in_maps.append({"zt": rot})
    return in_maps


def finish_host(results):
    """Assemble per-core row denominators into the scalar loss."""
    denom = np.empty(N2, dtype=np.float64)
    for c in range(NCORES):
        d = results[c]["dsum"].astype(np.float64)          # [128, ACOLS]
        # row (t*128 + p) local = global c*1024 + t*128 + p
        rows = np.zeros((P, MT))
        for col, t in enumerate(ACC_T):
            rows[:, t] += d[:, col]
        denom[c * ROWS : (c + 1) * ROWS] = rows.T.reshape(ROWS)
    denom -= np.exp(INV_T)                                 # drop diagonal term
    pos = _CACHE["pos"]
    loss = np.log(denom) - INV_T * np.concatenate([pos, pos])
    return np.float32(loss.sum() / N2)


def kernel(emb_i, emb_j):
    from concourse.bass_utils import run_bass_kernel_spmd

    nc = _get_nc()
    in_maps = make_in_maps(np.asarray(emb_i), np.asarray(emb_j))
    try:
        res = run_bass_kernel_spmd(nc, in_maps, core_ids=list(range(NCORES)))
    except Exception:
        res = run_bass_kernel_spmd(nc, in_maps, core_ids=list(range(NCORES)))
    _CACHE["last_results"] = res
    return finish_host(res.results)
